# revision 1
# baseline (speedup 1.0000x reference)
"""Trainium2 Bass kernel for nn_BidPrefix (segment_reduce).

Reference semantics, per row r (B=65536 rows, S=512 cols):
    cp[k]    = prod(x[r, 0:k])                  (exclusive prefix product)
    survival = cp[bid]
    rate     = cp[mp] - cp[mp+1], or EPS when mp == 0
returned as (survival [B,1] f32, rate_last [B,1] f32).

Design: masked products -- no cumprod materialisation, no dynamic
gather.  For each needed index k:
    cp[k] = reduce_mult_t( (t >= k) ? 1.0 : x[t] )
          = reduce_mult( max(x, (iota >= k)) )
The blend is ONE fused DVE scalar_tensor_tensor per (row-group, k):
    out = (iota cmp k[p]) max x    (cmp is_ge for k=bid,mp; is_gt for mp+1)
The {0,1} mask makes masked lanes contribute exactly 1.0 (multiplying by
1.0 is exact), so each product reproduces the reference's f32 prefix
product bit-for-bit, and k==0 (empty product == 1) needs no special
case.  All 3*G blends of a supertile land in one [128, 3G, 512] tile and
are reduced by a single 3D reduce_mult -> [128, 3G].

Supertile: [128, G*512] with row r = i*128*G + p*G + g (partition-major,
so each partition's DMA chunk is G*2KB contiguous).

Sharding: pure data parallel over the batch axis, B/8 = 8192 rows per
NeuronCore, same NEFF on all 8 cores (SPMD), outputs concatenated.
"""

import numpy as np

import concourse.bacc as bacc
import concourse.mybir as mybir
from concourse.tile import TileContext
from concourse.bass_utils import run_bass_kernel_spmd

f32 = mybir.dt.float32
i32 = mybir.dt.int32
Alu = mybir.AluOpType

N_CORES = 8
B, S = 65536, 512
ROWS = B // N_CORES          # 8192 rows per core
G = 8                        # 512-wide row-groups per supertile
SUPER = 128 * G              # 1024 rows per supertile
N_SUPER = ROWS // SUPER      # 8 supertiles per core
W = G * S
EPS = 1e-7


def build_bass():
    nc = bacc.Bacc()

    x = nc.dram_tensor("x", [ROWS, S], f32, kind="ExternalInput")
    bid_info = nc.dram_tensor("bid_info", [ROWS, 2], i32, kind="ExternalInput")
    surv_out = nc.dram_tensor("survival", [ROWS, 1], f32, kind="ExternalOutput")
    rate_out = nc.dram_tensor("rate_last", [ROWS, 1], f32, kind="ExternalOutput")

    x_v = x.rearrange("(i p g) s -> i p (g s)", p=128, g=G)
    bi_v = bid_info.rearrange("(i p g) c -> i p (g c)", p=128, g=G)
    so_v = surv_out.rearrange("(i p g) c -> i p (g c)", p=128, g=G)
    ro_v = rate_out.rearrange("(i p g) c -> i p (g c)", p=128, g=G)

    with TileContext(nc) as tc:
        with (
            tc.tile_pool(name="const", bufs=1) as cpool,
            tc.tile_pool(name="big", bufs=2) as bpool,
            tc.tile_pool(name="small", bufs=3) as spool,
        ):
            it512i = cpool.tile([128, 512], i32, tag="it512i")
            nc.gpsimd.iota(it512i[:], pattern=[[1, 512]], base=0,
                           channel_multiplier=0)
            it512 = cpool.tile([128, 512], f32, tag="it512")
            nc.vector.tensor_copy(out=it512[:], in_=it512i[:])

            for i in range(N_SUPER):
                xt = bpool.tile([128, W], f32, tag="xt")
                nc.sync.dma_start(out=xt[:], in_=x_v[i])
                bi = spool.tile([128, 2 * G], i32, tag="bi")
                nc.sync.dma_start(out=bi[:], in_=bi_v[i])

                bif = spool.tile([128, 2 * G], f32, tag="bif")
                nc.vector.tensor_copy(out=bif[:], in_=bi[:])
                bif3 = bif[:].rearrange("p (g c) -> p g c", c=2)
                mpf = bif3[:, :, 0:1]    # [128, G, 1] market price
                bidf = bif3[:, :, 1:2]   # [128, G, 1] bid

                # Tiny reads absorb the HWDGE per-queue semaphores before
                # the TensorScalarPtr-encoded STTs (that ISA encoding has
                # too few sync-wait slots to carry them itself).
                sink = spool.tile([128, 2], f32, tag="sink")
                nc.vector.tensor_copy(out=sink[:, 0:1], in_=xt[:, 0:1])

                # blends: BL[:, g*3+j, :]  j=0: k=bid, 1: k=mp, 2: k=mp+1
                BL = bpool.tile([128, 3 * G, S], f32, tag="BL")
                for g in range(G):
                    xg = xt[:, g * S:(g + 1) * S]
                    specs = [
                        (bidf[:, g, :], Alu.is_ge),
                        (mpf[:, g, :], Alu.is_ge),
                        (mpf[:, g, :], Alu.is_gt),
                    ]
                    for j, (kap, cmp) in enumerate(specs):
                        nc.vector.scalar_tensor_tensor(
                            out=BL[:, g * 3 + j, :], in0=it512[:],
                            scalar=kap, in1=xg, op0=cmp, op1=Alu.max)

                raw = spool.tile([128, 3 * G], f32, tag="raw")
                nc.vector.tensor_reduce(out=raw[:], in_=BL[:],
                                        axis=mybir.AxisListType.X, op=Alu.mult)
                raw3 = raw[:].rearrange("p (g j) -> p g j", j=3)
                svraw = raw3[:, :, 0]
                g1raw = raw3[:, :, 1]
                g2raw = raw3[:, :, 2]

                # rate = (g1-g2)*(1-(mp==0)) + EPS*(mp==0)  -- exact select
                m0m = spool.tile([128, G], f32, tag="m0m")
                nc.vector.tensor_scalar(out=m0m[:], in0=mpf, scalar1=0.0,
                                        scalar2=None, op0=Alu.is_equal)
                onem = spool.tile([128, G], f32, tag="onem")
                nc.vector.tensor_scalar(out=onem[:], in0=m0m[:], scalar1=-1.0,
                                        scalar2=1.0, op0=Alu.mult, op1=Alu.add)
                rate0 = spool.tile([128, G], f32, tag="rate0")
                nc.vector.tensor_sub(out=rate0[:], in0=g1raw, in1=g2raw)
                rate1 = spool.tile([128, G], f32, tag="rate1")
                nc.vector.tensor_mul(out=rate1[:], in0=rate0[:], in1=onem[:])
                rate_t = spool.tile([128, G], f32, tag="rate_t")
                nc.vector.scalar_tensor_tensor(
                    out=rate_t[:], in0=m0m[:], scalar=EPS, in1=rate1[:],
                    op0=Alu.mult, op1=Alu.add)

                nc.sync.dma_start(out=so_v[i], in_=svraw)
                nc.sync.dma_start(out=ro_v[i], in_=rate_t[:])
    nc.finalize()
    return nc


_NC_CACHE = None


def _get_nc():
    global _NC_CACHE
    if _NC_CACHE is None:
        _NC_CACHE = build_bass()
    return _NC_CACHE


def kernel(x, bid_info):
    x = np.ascontiguousarray(np.asarray(x, dtype=np.float32))
    bid_info = np.ascontiguousarray(np.asarray(bid_info, dtype=np.int32))
    assert x.shape == (B, S) and bid_info.shape == (B, 2)

    nc = _get_nc()
    in_maps = [
        {
            "x": x[c * ROWS:(c + 1) * ROWS],
            "bid_info": bid_info[c * ROWS:(c + 1) * ROWS],
        }
        for c in range(N_CORES)
    ]
    res = run_bass_kernel_spmd(nc, in_maps, core_ids=list(range(N_CORES)))
    survival = np.concatenate([r["survival"] for r in res.results], axis=0)
    rate_last = np.concatenate([r["rate_last"] for r in res.results], axis=0)
    return survival, rate_last



# revision 21
# speedup vs baseline: 1.8795x; 1.8795x over previous
"""Trainium2 Bass kernel for nn_BidPrefix (segment_reduce).

Reference semantics, per row r (B=65536 rows, S=512 cols):
    cp[k]    = prod(x[r, 0:k])                  (exclusive prefix product)
    survival = cp[bid]
    rate     = cp[mp] - cp[mp+1], or EPS when mp == 0
returned as (survival [B,1] f32, rate_last [B,1] f32).

Design: log-domain fused masked sums.
    ln cp[k] = sum_t (iota[t] < k) * ln(x[t])
The Activation engine computes L = ln(x + 1e-38) once (fp16).  Per
row-group on DVE:
  * scalar_tensor_tensor with fused add-accumulator:
        s_bid = sum((iota is_lt bid) * L) = ln cp[bid]; same for mp
  * tensor_mask_reduce with a width-1 window [mp, mp+1) and op=max is
    a true per-row gather of L[mp] (2x mode on fp16);
    cp[mp+1] = cp[mp] * e^{L[mp]} so no third masked sum is needed.
(The Pool engine cannot run TensorScalarPtr -- the ISA check rejects
it -- so all masked sums live on DVE.)
Epilogue: exp on ACT, rate = e^{s_mp} * (1 - e^{L[mp]}) * [mp != 0]
+ EPS * [mp == 0] (exact select).  All Ln activations precede all Exp
activations so each activation table loads once; a dummy Ln up front
hides the first load inside the DMA ramp.

Numerics: L >= ln(1e-38) = -87.5 (no inf/NaN), fp16 L carries <= 4.9e-4
relative error per element, so worst scale-relative output error is
~ max_k sqrt(k) e^-k * 5e-4 ~ 2e-4, far inside the 2e-2 gate.

Supertile: [128, G*512] with row r = i*128*G + p*G + g (partition-major:
each partition's DMA chunk is G*2KB contiguous).  The first supertiles
are split into smaller chunks so compute engines ramp up sooner.
bid_info is fetched in ONE DMA right after the first x chunk; outputs
are staged in SBUF and stored in one DMA per half.

Sharding: pure data parallel over the batch axis, B/8 = 8192 rows per
NeuronCore, same NEFF on all 8 cores (SPMD), outputs concatenated.
"""

import numpy as np

import concourse.bacc as bacc
import concourse.mybir as mybir
from concourse.tile import TileContext
from concourse.bass_utils import run_bass_kernel_spmd

f32 = mybir.dt.float32
f16 = mybir.dt.float16
i32 = mybir.dt.int32
Alu = mybir.AluOpType
Act = mybir.ActivationFunctionType

N_CORES = 8
B, S = 65536, 512
ROWS = B // N_CORES          # 8192 rows per core
G = 4                        # 512-wide row-groups per supertile
SUPER = 128 * G              # 512 rows per supertile
N_SUPER = ROWS // SUPER      # 16 supertiles per core
W = G * S
NK = N_SUPER * G             # 64 row-group columns per partition
EPS = 1e-7
NEG_BIG = -3.0e38            # accum_in seed for max-gather
HALF = NK // 2               # epilogue half split
# tensor_mask_reduce compiles but crashes the exec unit at runtime on this
# stack (bisect-verified), so the third masked sum uses an is_le STT instead.
USE_TMR = False


def build_bass():
    nc = bacc.Bacc()

    x = nc.dram_tensor("x", [ROWS, S], f32, kind="ExternalInput")
    bid_info = nc.dram_tensor("bid_info", [ROWS, 2], i32, kind="ExternalInput")
    surv_out = nc.dram_tensor("survival", [ROWS, 1], f32, kind="ExternalOutput")
    rate_out = nc.dram_tensor("rate_last", [ROWS, 1], f32, kind="ExternalOutput")

    x_v = x.rearrange("(i p g) s -> i p (g s)", p=128, g=G)
    bi_v = bid_info.rearrange("(i p g) c -> p i (g c)", p=128, g=G)
    so_v = surv_out.rearrange("(i p g) c -> p i (g c)", p=128, g=G)
    ro_v = rate_out.rearrange("(i p g) c -> p i (g c)", p=128, g=G)

    with TileContext(nc) as tc:
        with (
            tc.tile_pool(name="const", bufs=1) as cpool,
            tc.tile_pool(name="xbuf", bufs=4) as xpool,
            tc.tile_pool(name="lbuf", bufs=5) as lpool,
            tc.tile_pool(name="scr_d", bufs=2) as dpool,
        ):
            # ln bias (guards ln(0) -> -inf; 1e-38 leaves normal x unchanged)
            lnb = cpool.tile([128, 1], f32, tag="lnb")
            nc.vector.memset(lnb[:], 1e-38)

            # dummy Ln: pulls the natural_log table load into the DMA ramp
            warm = cpool.tile([128, 1], f32, tag="warm")
            nc.scalar.activation(out=warm[:], in_=lnb[:], func=Act.Ln)

            # fp16 iota 0..511 (integers <= 2048 are exact in fp16)
            it_i = cpool.tile([128, S], i32, tag="it_i")
            nc.gpsimd.iota(it_i[:], pattern=[[1, S]], base=0,
                           channel_multiplier=0)
            it_f = cpool.tile([128, S], f32, tag="it_f")
            nc.vector.tensor_copy(out=it_f[:], in_=it_i[:])
            it_h = cpool.tile([128, S], f16, tag="it_h")
            nc.vector.tensor_copy(out=it_h[:], in_=it_f[:])

            bi = cpool.tile([128, NK * 2], i32, tag="bi")
            bif = cpool.tile([128, NK * 2], f32, tag="bif")
            mp_pk = cpool.tile([128, NK], f32, tag="mp_pk")
            mpp1 = cpool.tile([128, NK], f32, tag="mpp1")
            m0 = cpool.tile([128, NK], f32, tag="m0")
            onem = cpool.tile([128, NK], f32, tag="onem")

            # masked log-sum accumulators; l_mp holds the gathered L[mp]
            # (USE_TMR) or the s_mp1 = ln cp[mp+1] masked sum (third STT)
            s_bid = cpool.tile([128, NK], f32, tag="s_bid")
            s_mp = cpool.tile([128, NK], f32, tag="s_mp")
            l_mp = cpool.tile([128, NK], f32, tag="l_mp")

            # output staging
            e_bid = cpool.tile([128, NK], f32, tag="e_bid")
            e_mp = cpool.tile([128, NK], f32, tag="e_mp")
            e_lmp = cpool.tile([128, NK], f32, tag="e_lmp")
            one_x = cpool.tile([128, NK], f32, tag="one_x")
            u_t = cpool.tile([128, NK], f32, tag="u_t")
            rate1 = cpool.tile([128, NK], f32, tag="rate1")
            rate_t = cpool.tile([128, NK], f32, tag="rate_t")

            def epilogue_half(h):
                lo, hi = h * HALF, (h + 1) * HALF
                sl = slice(lo, hi)
                # e_lmp leads: it depends on the last DVE producer, the
                # longest dependency chain; e_bid (surv) trails.
                nc.scalar.activation(out=e_lmp[:, sl], in_=l_mp[:, sl],
                                     func=Act.Exp)
                nc.scalar.activation(out=e_mp[:, sl], in_=s_mp[:, sl],
                                     func=Act.Exp)
                nc.scalar.activation(out=e_bid[:, sl], in_=s_bid[:, sl],
                                     func=Act.Exp)
                if USE_TMR:
                    # e_lmp = e^{L[mp]}: rate1 = e_mp*(1-e_lmp)*[mp!=0]
                    nc.scalar.activation(out=one_x[:, sl], in_=e_lmp[:, sl],
                                         func=Act.Copy, bias=1.0, scale=-1.0)
                    nc.vector.tensor_mul(out=u_t[:, sl], in0=one_x[:, sl],
                                         in1=onem[:, sl])
                    nc.vector.tensor_mul(out=rate1[:, sl], in0=e_mp[:, sl],
                                         in1=u_t[:, sl])
                else:
                    # e_lmp = cp[mp+1]: rate1 = (e_mp - e_lmp)*[mp!=0]
                    nc.vector.tensor_sub(out=u_t[:, sl], in0=e_mp[:, sl],
                                         in1=e_lmp[:, sl])
                    nc.vector.tensor_mul(out=rate1[:, sl], in0=u_t[:, sl],
                                         in1=onem[:, sl])
                nc.vector.scalar_tensor_tensor(
                    out=rate_t[:, sl], in0=m0[:, sl], scalar=EPS,
                    in1=rate1[:, sl], op0=Alu.mult, op1=Alu.add)
                ih = slice(h * (N_SUPER // 2), (h + 1) * (N_SUPER // 2))
                nc.sync.dma_start(
                    out=so_v[:, ih],
                    in_=e_bid[:, sl].rearrange("p (i g) -> p i g",
                                               i=N_SUPER // 2))
                nc.sync.dma_start(
                    out=ro_v[:, ih],
                    in_=rate_t[:, sl].rearrange("p (i g) -> p i g",
                                                i=N_SUPER // 2))

            # ramp: split the first supertiles into small chunks so the first
            # ln lands on ACT sooner and DVE doesn't starve at start.
            schedule = [(0, g, 1) for g in range(G)]
            schedule += [(1, 0, 2), (1, 2, 2)]
            schedule += [(i, 0, G) for i in range(2, N_SUPER)]

            first = True
            for (i, g0, gn) in schedule:
                wid = gn * S
                xt = xpool.tile([128, wid], f32, tag=f"xt{gn}")
                nc.sync.dma_start(out=xt[:],
                                  in_=x_v[i][:, g0 * S:(g0 + gn) * S])
                if first:
                    # bid_info DMA rides right behind the first x chunk
                    nc.sync.dma_start(
                        out=bi[:].rearrange("p (i k) -> p i k", i=N_SUPER),
                        in_=bi_v)

                lt = lpool.tile([128, wid], f16, tag=f"lt{gn}")
                nc.scalar.activation(out=lt[:], in_=xt[:], func=Act.Ln,
                                     bias=lnb[:])

                if first:
                    # scalar conversions on DVE, which idles until the first
                    # ln lands; m0/onem are epilogue-only and come last
                    nc.vector.tensor_copy(out=bif[:], in_=bi[:])
                    nc.vector.tensor_copy(
                        out=mp_pk[:],
                        in_=bif[:].rearrange("p (n c) -> p n c", c=2)[:, :, 0])
                    nc.vector.tensor_scalar(out=mpp1[:], in0=mp_pk[:],
                                            scalar1=1.0, scalar2=None,
                                            op0=Alu.add)
                    nc.vector.tensor_scalar(out=m0[:], in0=mp_pk[:],
                                            scalar1=0.0, scalar2=None,
                                            op0=Alu.is_equal)
                    nc.vector.tensor_scalar(out=onem[:], in0=m0[:],
                                            scalar1=-1.0, scalar2=1.0,
                                            op0=Alu.mult, op1=Alu.add)
                    first = False

                # on the last supertile, emit all gathers first and the s_mp
                # sums last so the epilogue's longest dependency chains start
                # as early as possible while the final STTs still stream.
                if i == N_SUPER - 1:
                    phases = ("gather", "bid", "mp")
                else:
                    phases = ("all",)
                for phase in phases:
                    for g in range(g0, g0 + gn):
                        col = i * G + g
                        lg = lt[:, (g - g0) * S:(g - g0 + 1) * S]
                        mp_ap = bif[:, 2 * col:2 * col + 1]
                        bid_ap = bif[:, 2 * col + 1:2 * col + 2]

                        if phase in ("all", "gather"):
                            scrm = dpool.tile([128, S], f16, tag="scrm")
                            if USE_TMR:
                                # gather L[mp] via width-1 window max-reduce
                                nc.vector.tensor_mask_reduce(
                                    out=scrm[:], in_=lg,
                                    mask_start=mp_pk[:, col:col + 1],
                                    mask_end=mpp1[:, col:col + 1],
                                    scale=1.0, accum_in=NEG_BIG, op=Alu.max,
                                    accum_out=l_mp[:, col:col + 1])
                            else:
                                # s_mp1 = sum L[t], t <= mp  (= ln cp[mp+1])
                                nc.vector.scalar_tensor_tensor(
                                    out=scrm[:], in0=it_h[:], scalar=mp_ap,
                                    in1=lg, op0=Alu.is_le, op1=Alu.mult,
                                    accum_out=l_mp[:, col:col + 1])

                        if phase in ("all", "bid"):
                            scr = dpool.tile([128, S], f16, tag="scr")
                            nc.vector.scalar_tensor_tensor(
                                out=scr[:], in0=it_h[:], scalar=bid_ap, in1=lg,
                                op0=Alu.is_lt, op1=Alu.mult,
                                accum_out=s_bid[:, col:col + 1])

                        if phase in ("all", "mp"):
                            scr2 = dpool.tile([128, S], f16, tag="scr2")
                            nc.vector.scalar_tensor_tensor(
                                out=scr2[:], in0=it_h[:], scalar=mp_ap, in1=lg,
                                op0=Alu.is_lt, op1=Alu.mult,
                                accum_out=s_mp[:, col:col + 1])

            # both epilogue halves sit after every Ln in ACT program order:
            # one Exp table load, overlapped with the last supertiles' STTs.
            epilogue_half(0)
            epilogue_half(1)
    nc.finalize()
    return nc


_NC_CACHE = None


def _get_nc():
    global _NC_CACHE
    if _NC_CACHE is None:
        _NC_CACHE = build_bass()
    return _NC_CACHE


def kernel(x, bid_info):
    x = np.ascontiguousarray(np.asarray(x, dtype=np.float32))
    bid_info = np.ascontiguousarray(np.asarray(bid_info, dtype=np.int32))
    assert x.shape == (B, S) and bid_info.shape == (B, 2)

    nc = _get_nc()
    in_maps = [
        {
            "x": x[c * ROWS:(c + 1) * ROWS],
            "bid_info": bid_info[c * ROWS:(c + 1) * ROWS],
        }
        for c in range(N_CORES)
    ]
    res = run_bass_kernel_spmd(nc, in_maps, core_ids=list(range(N_CORES)))
    survival = np.concatenate([r["survival"] for r in res.results], axis=0)
    rate_last = np.concatenate([r["rate_last"] for r in res.results], axis=0)
    return survival, rate_last
